# revision 1
# baseline (speedup 1.0000x reference)
"""Trainium2 Bass kernel for BasicLSTM (B=64, T=512, D=U=512).

Sharding: data-parallel over batch across 8 cores (8 rows/core), weights
replicated; the sequential time scan runs locally per core.

Per-core strategy (everything unit-major / "transposed", all-SBUF):
  Phase A: zx.T = Wk.T @ x.T + b computed directly in unit-major layout.
    x is loaded with fast contiguous DMAs, converted to bf16, transposed
    on-chip via the DMA xbar (dedicated queue), then used as the moving
    operand against stationary bf16 Wk tiles.  Bias is applied via the ACT
    per-partition bias during PSUM->SBUF copy-out.  The whole
    zx.T [128p, T*(16m*8b)] stays resident in SBUF as bf16 (16 MB).
  Phase B: 512-step scan with zero DMA.  Gate m-tiles are reordered
    [i,f,o,g] and the 16 m-tiles are processed in two halves, each into its
    own PSUM bank, so the elementwise tail of the first half overlaps the
    matmuls of the second:
      z.T[t] = sum_k Wr[k,m].T @ h.T[k]      (PE, bf16+FWL, 64 LDW+MM)
      psum += zx.T[t]                        (DVE, in place)
      i,f,o = sigmoid(psum), g = tanh(psum)  (ACT, reads PSUM)
      c' = f*c + i*g ; h' = o*tanh(c')       (DVE/ACT)
    h/c are split into per-half tiles; h is bf16 (feeds the next matmul),
    c stays fp32.  The final h is computed in fp32 and DMA'd out.
"""

import numpy as np

B, T, D, U = 64, 512, 512, 512
G = 4 * U            # gates
P = 128              # partitions
N_CORES = 8
B_LOC = B // N_CORES  # 8
KD = D // P          # 4 k-tiles for x@Wk
KU = U // P          # 4 k-tiles for h@Wr
M = G // P           # 16 m-tiles of gates
TC = 64              # timesteps per phase-A chunk
FB = M * B_LOC       # 128 free cols of z per step
HB = FB // 2         # 64 cols per half

# gate reordering: new m-tile order [i, f, o, g] -> original m-tile index
PERMM = list(range(8)) + [12, 13, 14, 15] + [8, 9, 10, 11]
# halves: half h holds m-tiles {4a + q : a in 0..3} for q in {2h, 2h+1}
HALF_MS = [[0, 4, 8, 12, 1, 5, 9, 13], [2, 6, 10, 14, 3, 7, 11, 15]]

_CACHE = {}


def _build(time_steps=T):
    import concourse.bacc as bacc
    import concourse.tile as tile
    import concourse.mybir as mybir
    from bass_rust import add_dep_helper

    f32 = mybir.dt.float32
    bf16 = mybir.dt.bfloat16
    AF = mybir.ActivationFunctionType

    nc = bacc.Bacc(
        "TRN2",
        target_bir_lowering=False,
        debug=False,
        enable_asserts=True,
        num_devices=N_CORES,
    )

    x_h = nc.dram_tensor("x", [B_LOC, T, D], f32, kind="ExternalInput")
    wk_h = nc.dram_tensor("Wk", [D, G], f32, kind="ExternalInput")
    wr_h = nc.dram_tensor("Wr", [U, G], f32, kind="ExternalInput")
    b_h = nc.dram_tensor("b", [G], f32, kind="ExternalInput")
    out_h = nc.dram_tensor("h_last", [B_LOC, U], f32, kind="ExternalOutput")

    x_ap = x_h.ap()

    def load_weight_bf16(dst, src_h, stage_pool):
        """[512, 2048] fp32 weight -> dst bf16 [128, 64*128] laid out as
        (k, new_m) tiles of [128, 128] with the [i,f,o,g] gate reorder."""
        for k in range(KD):
            st = stage_pool.tile([P, G], f32, name="wstage", tag="wstage")
            nc.gpsimd.dma_start(st[:], src_h.ap()[k * P:(k + 1) * P, :])
            for nm0, om0, w in ((0, 0, 8), (8, 12, 4), (12, 8, 4)):
                nc.vector.tensor_copy(
                    dst[:, (k * M + nm0) * P:(k * M + nm0 + w) * P],
                    st[:, om0 * P:(om0 + w) * P],
                )

    with tile.TileContext(nc) as tc:
        with (
            tc.tile_pool(name="persist", bufs=1) as persist_pool,
        ):
            # zx.T resident in SBUF: col = m*(T*8) + b*64 + t  (bf16, 128KB/par)
            # (phase A writes [128, 512] contiguous per (m, chunk); the scan
            #  reads a strided comb per step, which is free on DVE)
            zxT = persist_pool.tile([P, T * FB], bf16)
            zxT4 = zxT.rearrange("p (m b t) -> p m b t", m=M, b=B_LOC)
            b_sb = persist_pool.tile([P, M], f32)
            nc.sync.dma_start(b_sb[:], b_h.ap().rearrange("(m p) -> p m", p=P))

            # ---------------- Phase A: zx.T = Wk.T @ x.T + b ----------------
            with (
                tc.tile_pool(name="wk", bufs=1) as wk_pool,
                tc.tile_pool(name="stage", bufs=2) as stage_pool,
                tc.tile_pool(name="nat", bufs=2) as nat_pool,
                tc.tile_pool(name="xtb", bufs=2) as xtb_pool,
                tc.tile_pool(name="gemm_psum", bufs=4, space="PSUM") as gps_pool,
            ):
                wk_sb = wk_pool.tile([P, KD * G], bf16)
                load_weight_bf16(wk_sb, wk_h, stage_pool)

                for chunk in range(T // TC):
                    t0 = chunk * TC
                    # natural x loads: tile bp holds rows (b=2bp..2bp+1, t0..t0+63)
                    natbs = []
                    for bp in range(4):
                        nat = nat_pool.tile([P, D], f32, name="nat", tag=f"nat{bp}")
                        for j in range(2):
                            nc.gpsimd.dma_start(
                                nat[j * TC:(j + 1) * TC, :],
                                x_ap[2 * bp + j, t0:t0 + TC, :],
                            )
                        natb = nat_pool.tile([P, D], bf16, name="natb", tag=f"natb{bp}")
                        nc.vector.tensor_copy(natb[:], nat[:])
                        natbs.append(natb)
                    # xbar transposes: xtb[k] cols = b*64 + t  (b-major)
                    xtbs = []
                    for k in range(KD):
                        xtb = xtb_pool.tile([P, TC * B_LOC], bf16,
                                            name=f"xtb{k}", tag=f"xtb{k}")
                        for bp in range(4):
                            nc.sync.dma_start(
                                xtb[:, bp * P:(bp + 1) * P],
                                natbs[bp][:, k * P:(k + 1) * P],
                                transpose=True,
                            )
                        xtbs.append(xtb)
                    for m in range(M):
                        ps = gps_pool.tile([P, TC * B_LOC], f32,
                                           name="gps", tag="gps")
                        for k in range(KD):
                            nc.tensor.matmul(
                                ps[:],
                                wk_sb[:, (k * M + m) * P:(k * M + m + 1) * P],
                                xtbs[k][:],
                                start=(k == 0),
                                stop=(k == KD - 1),
                            )
                        # copy-out + per-partition bias
                        # psum free = (b, t) b-major = contiguous dst slice
                        nc.scalar.activation(
                            zxT4[:, m, :, t0:t0 + TC],
                            ps.rearrange("p (b t) -> p b t", t=TC)[:],
                            AF.Identity,
                            bias=b_sb[:, PERMM[m]:PERMM[m] + 1],
                        )

            # ---------------- Phase B: the scan ----------------
            with (
                tc.tile_pool(name="wr", bufs=1) as wr_pool,
                tc.tile_pool(name="wstage2", bufs=2) as wstage2_pool,
                tc.tile_pool(name="state", bufs=1) as st_pool,
                tc.tile_pool(name="gates", bufs=2) as gate_pool,
                tc.tile_pool(name="tmp", bufs=2) as tmp_pool,
                tc.tile_pool(name="scan_psum", bufs=2, space="PSUM") as sps_pool,
            ):
                wr_sb = wr_pool.tile([P, KU * G], bf16)
                load_weight_bf16(wr_sb, wr_h, wstage2_pool)

                # h: bf16 per (parity, half); c: fp32 per (parity, half)
                hs = [[st_pool.tile([P, 2 * B_LOC], bf16, name=f"h{i}{j}")
                       for j in range(2)] for i in range(2)]
                cs = [[st_pool.tile([P, 2 * B_LOC], f32, name=f"c{i}{j}")
                       for j in range(2)] for i in range(2)]
                for j in range(2):
                    nc.vector.memset(hs[0][j][:], 0.0)
                    nc.vector.memset(cs[0][j][:], 0.0)
                hf = st_pool.tile([P, KU * B_LOC], f32, name="hf")

                # psum half tile col layout: a*16 + q*8 + b, a = gate class
                for t in range(time_steps):
                    pp = t % 2
                    qq = 1 - pp
                    h_prev = hs[pp]
                    pss = [sps_pool.tile([P, HB], f32, name=f"ps{hf_}",
                                         tag=f"ps{hf_}") for hf_ in range(2)]
                    # MM order: [half0 kk{0,1}], [half0 kk{2,3}],
                    #           [half1 kk{0,1}], [half1 kk{2,3}]
                    # - the first 16 pairs only need h half 0 (overlap with the
                    #   previous step's half-1 tail)
                    # - ps0 is complete after 32 pairs, so its tail starts at
                    #   the PE block's midpoint
                    # PSUM accumulation relies on per-element has_written:
                    # start=True only on the first MM per bank.
                    for half in range(2):
                        firstmm = True
                        for kpair in range(2):
                            for m in HALF_MS[half]:
                                a, q = m // 4, m % 4 - 2 * half
                                dst = pss[half][:, a * 16 + q * 8:
                                                a * 16 + q * 8 + 8]
                                for kk in (2 * kpair, 2 * kpair + 1):
                                    nc.tensor.matmul(
                                        dst,
                                        wr_sb[:, (kk * M + m) * P:
                                              (kk * M + m + 1) * P],
                                        h_prev[kk // 2][:, (kk % 2) * B_LOC:
                                                        (kk % 2 + 1) * B_LOC],
                                        start=firstmm,
                                        stop=(kpair == 1 and kk == KU - 1
                                              and m == HALF_MS[half][-1]),
                                        skip_group_check=True,
                                    )
                                    firstmm = False
                    last = t == time_steps - 1
                    prev_tc = None
                    prev_hmul = None
                    for half in range(2):
                        ps = pss[half]
                        # zx comb for this half: m = 4a + q + 2*half, all b,
                        # one t element each
                        zxh = (zxT4
                               .rearrange("p (a qq) b t -> p a qq b t", qq=4)
                               [:, :, 2 * half:2 * half + 2, :, t])
                        ps4 = ps.rearrange("p (a q b) -> p a q b", q=2, b=B_LOC)
                        i_zadd = nc.vector.tensor_add(ps4[:], ps4[:], zxh)
                        gt = gate_pool.tile([P, HB], f32, name=f"gt{half}",
                                            tag=f"gt{half}")
                        i_sig = nc.scalar.activation(gt[:, 0:48], ps[:, 0:48],
                                                     AF.Sigmoid)
                        nc.scalar.activation(gt[:, 48:64], ps[:, 48:64], AF.Tanh)
                        t1 = tmp_pool.tile([P, 2 * B_LOC], f32,
                                           name=f"t1{half}", tag=f"t1{half}")
                        nc.vector.tensor_mul(t1[:], gt[:, 16:32], cs[pp][half][:])
                        t2 = tmp_pool.tile([P, 2 * B_LOC], f32,
                                           name=f"t2{half}", tag=f"t2{half}")
                        nc.vector.tensor_mul(t2[:], gt[:, 0:16], gt[:, 48:64])
                        nc.vector.tensor_add(cs[qq][half][:], t1[:], t2[:])
                        tc_t = tmp_pool.tile([P, 2 * B_LOC], f32,
                                             name=f"tc{half}", tag=f"tc{half}")
                        i_tc = nc.scalar.activation(tc_t[:], cs[qq][half][:],
                                                    AF.Tanh)
                        if last:
                            i_hmul = nc.vector.tensor_mul(
                                hf[:, half * 16:(half + 1) * 16],
                                gt[:, 32:48], tc_t[:],
                            )
                        else:
                            i_hmul = nc.vector.tensor_mul(hs[qq][half][:],
                                                          gt[:, 32:48], tc_t[:])
                        if half == 1 and prev_tc is not None:
                            # keep ACT/DVE focused on the half-0 chain: half-1
                            # tail slots in only once half 0's h is produced
                            add_dep_helper(i_sig.ins, prev_tc.ins,
                                           reason="tail1 ACT after tail0 tanh_c")
                            add_dep_helper(i_zadd.ins, prev_hmul.ins,
                                           reason="tail1 zadd after tail0 h")
                        prev_tc, prev_hmul = i_tc, i_hmul

                for kk in range(KU):
                    nc.sync.dma_start(
                        out_h.ap()[:, kk * P:(kk + 1) * P].rearrange("b p -> p b"),
                        hf[:, kk * B_LOC:(kk + 1) * B_LOC],
                    )

    nc.compile()
    return nc


def _get_nc(time_steps=T):
    key = time_steps
    if key not in _CACHE:
        _CACHE[key] = _build(time_steps)
    return _CACHE[key]


def kernel(x, Wk, Wr, b):
    from concourse import bass_utils

    x = np.ascontiguousarray(np.asarray(x, dtype=np.float32))
    Wk = np.ascontiguousarray(np.asarray(Wk, dtype=np.float32))
    Wr = np.ascontiguousarray(np.asarray(Wr, dtype=np.float32))
    b = np.ascontiguousarray(np.asarray(b, dtype=np.float32))

    nc = _get_nc(T)
    in_maps = [
        {
            "x": x[c * B_LOC:(c + 1) * B_LOC],
            "Wk": Wk,
            "Wr": Wr,
            "b": b,
        }
        for c in range(N_CORES)
    ]
    res = bass_utils.run_bass_kernel_spmd(nc, in_maps, core_ids=list(range(N_CORES)))
    return np.concatenate([res.results[c]["h_last"] for c in range(N_CORES)], axis=0)



# revision 2
# speedup vs baseline: 1.1442x; 1.1442x over previous
"""Trainium2 Bass kernel for BasicLSTM (B=64, T=512, D=U=512).

Sharding: data-parallel over batch across 8 cores (8 rows/core), weights
replicated; the sequential time scan runs locally per core.

Per-core strategy (everything unit-major / "transposed", all-SBUF):
  Phase A: zx.T = Wk.T @ x.T + b computed directly in unit-major layout.
    x is loaded with fast contiguous DMAs, converted to bf16, transposed
    on-chip via the DMA xbar (dedicated queue), then used as the moving
    operand against stationary bf16 Wk tiles.  Bias is applied via the ACT
    per-partition bias during PSUM->SBUF copy-out.  The whole
    zx.T [128p, T*(16m*8b)] stays resident in SBUF as bf16 (16 MB).
  Phase B: 512-step scan with zero DMA, structured to minimize the serial
    dependency chain per step:
      - zx[t] is injected into PSUM by an identity matmul (start=True), so
        no DVE add sits on the critical path.
      - MMs are kk-major: all kk{0,1} pairs (which need only h half0) run
        first, then kk{2,3} (h half1).  The late half's PSUM completes
        ~450ns after its h dependency instead of ~900ns.
      - g-gate m-tiles come first within each segment so tanh(g) runs on
        ACT while the i/f/o MMs still stream (off the critical chain).
      - f*c runs on the otherwise idle GPSIMD; i*g / c / h on DVE; only
        sigmoid(i,f,o) and tanh(c) remain on the ACT critical chain.
      - Per-engine program order matches expected data-ready order to
        avoid head-of-line blocking in the in-order queues.
"""

import numpy as np

B, T, D, U = 64, 512, 512, 512
G = 4 * U            # gates
P = 128              # partitions
N_CORES = 8
B_LOC = B // N_CORES  # 8
KD = D // P          # 4 k-tiles for x@Wk
KU = U // P          # 4 k-tiles for h@Wr
M = G // P           # 16 m-tiles of gates
TC = 64              # timesteps per phase-A chunk
FB = M * B_LOC       # 128 free cols of z per step
HB = FB // 2         # 64 cols per half

# gate reordering: new m-tile order [i, f, o, g] -> original m-tile index
PERMM = list(range(8)) + [12, 13, 14, 15] + [8, 9, 10, 11]
# half h holds m-tiles {4a + q + 2h : a in 0..3, q in 0..1}; g-class (a=3)
# first so tanh(g) can run while the i/f/o matmuls still stream.
HALF_MS = [[12, 13, 0, 1, 4, 5, 8, 9], [14, 15, 2, 3, 6, 7, 10, 11]]

_CACHE = {}


def _build(time_steps=T):
    import concourse.bacc as bacc
    import concourse.tile as tile
    import concourse.mybir as mybir
    from concourse import masks

    f32 = mybir.dt.float32
    bf16 = mybir.dt.bfloat16
    AF = mybir.ActivationFunctionType

    nc = bacc.Bacc(
        "TRN2",
        target_bir_lowering=False,
        debug=False,
        enable_asserts=True,
        num_devices=N_CORES,
    )

    x_h = nc.dram_tensor("x", [B_LOC, T, D], f32, kind="ExternalInput")
    wk_h = nc.dram_tensor("Wk", [D, G], f32, kind="ExternalInput")
    wr_h = nc.dram_tensor("Wr", [U, G], f32, kind="ExternalInput")
    b_h = nc.dram_tensor("b", [G], f32, kind="ExternalInput")
    out_h = nc.dram_tensor("h_last", [B_LOC, U], f32, kind="ExternalOutput")

    x_ap = x_h.ap()

    def load_weight_bf16(dst, src_h, stage_pool):
        """[512, 2048] fp32 weight -> dst bf16 [128, 64*128] laid out as
        (k, new_m) tiles of [128, 128] with the [i,f,o,g] gate reorder."""
        for k in range(KD):
            st = stage_pool.tile([P, G], f32, name="wstage", tag="wstage")
            nc.gpsimd.dma_start(st[:], src_h.ap()[k * P:(k + 1) * P, :])
            for nm0, om0, w in ((0, 0, 8), (8, 12, 4), (12, 8, 4)):
                nc.vector.tensor_copy(
                    dst[:, (k * M + nm0) * P:(k * M + nm0 + w) * P],
                    st[:, om0 * P:(om0 + w) * P],
                )

    with tile.TileContext(nc) as tc:
        with (
            tc.tile_pool(name="persist", bufs=1) as persist_pool,
        ):
            # zx.T resident in SBUF: col = m*(T*8) + b*64 + t  (bf16, 128KB/par)
            zxT = persist_pool.tile([P, T * FB], bf16)
            zxT4 = zxT.rearrange("p (m b t) -> p m b t", m=M, b=B_LOC)
            b_sb = persist_pool.tile([P, M], f32)
            nc.sync.dma_start(b_sb[:], b_h.ap().rearrange("(m p) -> p m", p=P))
            # bf16 identity for the zx->PSUM injection matmul
            ident = persist_pool.tile([P, P], bf16)
            masks.make_identity(nc, ident[:])

            # ---------------- Phase A: zx.T = Wk.T @ x.T + b ----------------
            with (
                tc.tile_pool(name="wk", bufs=1) as wk_pool,
                tc.tile_pool(name="stage", bufs=2) as stage_pool,
                tc.tile_pool(name="nat", bufs=2) as nat_pool,
                tc.tile_pool(name="xtb", bufs=2) as xtb_pool,
                tc.tile_pool(name="gemm_psum", bufs=4, space="PSUM") as gps_pool,
            ):
                wk_sb = wk_pool.tile([P, KD * G], bf16)
                load_weight_bf16(wk_sb, wk_h, stage_pool)

                for chunk in range(T // TC):
                    t0 = chunk * TC
                    # natural x loads: tile bp holds rows (b=2bp..2bp+1, t0..t0+63)
                    natbs = []
                    for bp in range(4):
                        nat = nat_pool.tile([P, D], f32, name="nat", tag=f"nat{bp}")
                        for j in range(2):
                            nc.gpsimd.dma_start(
                                nat[j * TC:(j + 1) * TC, :],
                                x_ap[2 * bp + j, t0:t0 + TC, :],
                            )
                        natb = nat_pool.tile([P, D], bf16, name="natb", tag=f"natb{bp}")
                        nc.vector.tensor_copy(natb[:], nat[:])
                        natbs.append(natb)
                    # xbar transposes: xtb[k] cols = b*64 + t  (b-major)
                    xtbs = []
                    for k in range(KD):
                        xtb = xtb_pool.tile([P, TC * B_LOC], bf16,
                                            name=f"xtb{k}", tag=f"xtb{k}")
                        for bp in range(4):
                            nc.sync.dma_start(
                                xtb[:, bp * P:(bp + 1) * P],
                                natbs[bp][:, k * P:(k + 1) * P],
                                transpose=True,
                            )
                        xtbs.append(xtb)
                    for m in range(M):
                        ps = gps_pool.tile([P, TC * B_LOC], f32,
                                           name="gps", tag="gps")
                        for k in range(KD):
                            nc.tensor.matmul(
                                ps[:],
                                wk_sb[:, (k * M + m) * P:(k * M + m + 1) * P],
                                xtbs[k][:],
                                start=(k == 0),
                                stop=(k == KD - 1),
                            )
                        # copy-out + per-partition bias
                        # psum free = (b, t) b-major = contiguous dst slice
                        nc.scalar.activation(
                            zxT4[:, m, :, t0:t0 + TC],
                            ps.rearrange("p (b t) -> p b t", t=TC)[:],
                            AF.Identity,
                            bias=b_sb[:, PERMM[m]:PERMM[m] + 1],
                        )

            # ---------------- Phase B: the scan ----------------
            with (
                tc.tile_pool(name="wr", bufs=1) as wr_pool,
                tc.tile_pool(name="wstage2", bufs=2) as wstage2_pool,
                tc.tile_pool(name="state", bufs=1) as st_pool,
                tc.tile_pool(name="gates", bufs=2) as gate_pool,
                tc.tile_pool(name="tmp", bufs=2) as tmp_pool,
                tc.tile_pool(name="scan_psum", bufs=4, space="PSUM") as sps_pool,
            ):
                wr_sb = wr_pool.tile([P, KU * G], bf16)
                load_weight_bf16(wr_sb, wr_h, wstage2_pool)

                # h: bf16 per (parity, half); c: fp32 per (parity, half)
                hs = [[st_pool.tile([P, 2 * B_LOC], bf16, name=f"h{i}{j}")
                       for j in range(2)] for i in range(2)]
                cs = [[st_pool.tile([P, 2 * B_LOC], f32, name=f"c{i}{j}")
                       for j in range(2)] for i in range(2)]
                for j in range(2):
                    nc.vector.memset(hs[0][j][:], 0.0)
                    nc.vector.memset(cs[0][j][:], 0.0)
                hf = st_pool.tile([P, KU * B_LOC], f32, name="hf")

                # psum half tile col layout: a*16 + q*8 + b, a = gate class
                for t in range(time_steps):
                    pp = t % 2
                    qq = 1 - pp
                    h_prev = hs[pp]
                    last = t == time_steps - 1

                    pss = [sps_pool.tile([P, HB], f32, name=f"ps{hf_}",
                                         tag=f"ps{hf_}") for hf_ in range(2)]
                    zxh = [
                        (zxT4
                         .rearrange("p (a qq) b t -> p a qq b t", qq=4)
                         [:, :, 2 * half:2 * half + 2, :, t])
                        for half in range(2)
                    ]
                    # zx injection: psum <- I @ zx (start=True), off-chain
                    for half in range(2):
                        nc.tensor.matmul(
                            pss[half].rearrange("p (a q b) -> p a q b",
                                                q=2, b=B_LOC)[:],
                            ident[:],
                            zxh[half],
                            start=True,
                            stop=False,
                            skip_group_check=True,
                        )
                    # weight MMs, kk-major: kk{0,1} (need h half0 only) first,
                    # then kk{2,3} (h half1).  g-class m-tiles lead each
                    # segment so tanh(g) can run early on ACT.
                    for kpair in range(2):
                        for half in range(2):
                            for m in HALF_MS[half]:
                                a, q = m // 4, m % 4 - 2 * half
                                dst = pss[half][:, a * 16 + q * 8:
                                                a * 16 + q * 8 + 8]
                                for kk in (2 * kpair, 2 * kpair + 1):
                                    nc.tensor.matmul(
                                        dst,
                                        wr_sb[:, (kk * M + m) * P:
                                              (kk * M + m + 1) * P],
                                        h_prev[kk // 2][:, (kk % 2) * B_LOC:
                                                        (kk % 2 + 1) * B_LOC],
                                        start=False,
                                        stop=(kpair == 1 and kk == KU - 1
                                              and m == HALF_MS[half][-1]),
                                        skip_group_check=True,
                                    )

                    # gates: ACT order = expected ready order
                    gts = [gate_pool.tile([P, HB], f32, name=f"gt{half}",
                                          tag=f"gt{half}") for half in range(2)]
                    nc.scalar.activation(gts[0][:, 48:64], pss[0][:, 48:64],
                                         AF.Tanh)
                    nc.scalar.activation(gts[0][:, 0:48], pss[0][:, 0:48],
                                         AF.Sigmoid)
                    nc.scalar.activation(gts[1][:, 48:64], pss[1][:, 48:64],
                                         AF.Tanh)
                    nc.scalar.activation(gts[1][:, 0:48], pss[1][:, 0:48],
                                         AF.Sigmoid)

                    t1s, t2s, tcs = [], [], []
                    for half in range(2):
                        gt = gts[half]
                        # f*c on GPSIMD (idle otherwise), i*g / c / h on DVE
                        t1 = tmp_pool.tile([P, 2 * B_LOC], f32,
                                           name=f"t1{half}", tag=f"t1{half}")
                        nc.gpsimd.tensor_mul(t1[:], gt[:, 16:32], cs[pp][half][:])
                        t2 = tmp_pool.tile([P, 2 * B_LOC], f32,
                                           name=f"t2{half}", tag=f"t2{half}")
                        nc.vector.tensor_mul(t2[:], gt[:, 0:16], gt[:, 48:64])
                        nc.vector.tensor_add(cs[qq][half][:], t1[:], t2[:])
                        tc_t = tmp_pool.tile([P, 2 * B_LOC], f32,
                                             name=f"tc{half}", tag=f"tc{half}")
                        nc.scalar.activation(tc_t[:], cs[qq][half][:], AF.Tanh)
                        t1s.append(t1)
                        t2s.append(t2)
                        tcs.append(tc_t)
                    for half in range(2):
                        if last:
                            nc.vector.tensor_mul(
                                hf[:, half * 16:(half + 1) * 16],
                                gts[half][:, 32:48], tcs[half][:],
                            )
                        else:
                            nc.vector.tensor_mul(hs[qq][half][:],
                                                 gts[half][:, 32:48],
                                                 tcs[half][:])

                for kk in range(KU):
                    nc.sync.dma_start(
                        out_h.ap()[:, kk * P:(kk + 1) * P].rearrange("b p -> p b"),
                        hf[:, kk * B_LOC:(kk + 1) * B_LOC],
                    )

    nc.compile()
    return nc


def _get_nc(time_steps=T):
    key = time_steps
    if key not in _CACHE:
        _CACHE[key] = _build(time_steps)
    return _CACHE[key]


def kernel(x, Wk, Wr, b):
    from concourse import bass_utils

    x = np.ascontiguousarray(np.asarray(x, dtype=np.float32))
    Wk = np.ascontiguousarray(np.asarray(Wk, dtype=np.float32))
    Wr = np.ascontiguousarray(np.asarray(Wr, dtype=np.float32))
    b = np.ascontiguousarray(np.asarray(b, dtype=np.float32))

    nc = _get_nc(T)
    in_maps = [
        {
            "x": x[c * B_LOC:(c + 1) * B_LOC],
            "Wk": Wk,
            "Wr": Wr,
            "b": b,
        }
        for c in range(N_CORES)
    ]
    res = bass_utils.run_bass_kernel_spmd(nc, in_maps, core_ids=list(range(N_CORES)))
    return np.concatenate([res.results[c]["h_last"] for c in range(N_CORES)], axis=0)


# revision 3
# speedup vs baseline: 1.1818x; 1.0329x over previous
"""Trainium2 Bass kernel for BasicLSTM (B=64, T=512, D=U=512).

Sharding: data-parallel over batch across 8 cores (8 rows/core), weights
replicated; the sequential time scan runs locally per core.

Per-core strategy (everything unit-major / "transposed", all-SBUF):
  Phase A: zx.T = Wk.T @ x.T + b computed directly in unit-major layout.
    x is loaded with fast contiguous DMAs, converted to bf16, transposed
    on-chip via the DMA xbar (dedicated queue), then used as the moving
    operand against stationary bf16 Wk tiles.  Bias is applied via the ACT
    per-partition bias during PSUM->SBUF copy-out.  The whole
    zx.T [128p, T*(16m*8b)] stays resident in SBUF as bf16 (16 MB).
  Phase B: 512-step scan with zero DMA, structured to minimize the serial
    dependency chain per step:
      - zx[t] is injected into PSUM by an identity matmul (start=True), so
        no DVE add sits on the critical path.
      - MMs are kk-major: all kk{0,1} pairs (which need only h half0) run
        first, then kk{2,3} (h half1).  The late half's PSUM completes
        ~450ns after its h dependency instead of ~900ns.
      - g-gate m-tiles come first within each segment so tanh(g) runs on
        ACT while the i/f/o MMs still stream (off the critical chain).
      - f*c runs on the otherwise idle GPSIMD; i*g / c / h on DVE; only
        sigmoid(i,f,o) and tanh(c) remain on the ACT critical chain.
      - Per-engine program order matches expected data-ready order to
        avoid head-of-line blocking in the in-order queues.
"""

import numpy as np

B, T, D, U = 64, 512, 512, 512
G = 4 * U            # gates
P = 128              # partitions
N_CORES = 8
B_LOC = B // N_CORES  # 8
KD = D // P          # 4 k-tiles for x@Wk
KU = U // P          # 4 k-tiles for h@Wr
M = G // P           # 16 m-tiles of gates
TC = 64              # timesteps per phase-A chunk
FB = M * B_LOC       # 128 free cols of z per step
HB = FB // 2         # 64 cols per half

# gate reordering: new m-tile order [i, f, o, g] -> original m-tile index
PERMM = list(range(8)) + [12, 13, 14, 15] + [8, 9, 10, 11]
# half h holds m-tiles {4a + q + 2h : a in 0..3, q in 0..1}; g-class (a=3)
# first so tanh(g) can run while the i/f/o matmuls still stream.
HALF_MS = [[12, 13, 0, 1, 4, 5, 8, 9], [14, 15, 2, 3, 6, 7, 10, 11]]

_CACHE = {}


def _build(time_steps=T):
    import concourse.bacc as bacc
    import concourse.tile as tile
    import concourse.mybir as mybir
    from concourse import masks

    f32 = mybir.dt.float32
    bf16 = mybir.dt.bfloat16
    AF = mybir.ActivationFunctionType

    nc = bacc.Bacc(
        "TRN2",
        target_bir_lowering=False,
        debug=False,
        enable_asserts=True,
        num_devices=N_CORES,
    )

    x_h = nc.dram_tensor("x", [B_LOC, T, D], f32, kind="ExternalInput")
    wk_h = nc.dram_tensor("Wk", [D, G], f32, kind="ExternalInput")
    wr_h = nc.dram_tensor("Wr", [U, G], f32, kind="ExternalInput")
    b_h = nc.dram_tensor("b", [G], f32, kind="ExternalInput")
    out_h = nc.dram_tensor("h_last", [B_LOC, U], f32, kind="ExternalOutput")

    x_ap = x_h.ap()

    def load_weight_bf16(dst, src_h, stage_pool):
        """[512, 2048] fp32 weight -> dst bf16 [128, 64*128] laid out as
        (k, new_m) tiles of [128, 128] with the [i,f,o,g] gate reorder."""
        for k in range(KD):
            st = stage_pool.tile([P, G], f32, name="wstage", tag="wstage")
            nc.gpsimd.dma_start(st[:], src_h.ap()[k * P:(k + 1) * P, :])
            for nm0, om0, w in ((0, 0, 8), (8, 12, 4), (12, 8, 4)):
                nc.vector.tensor_copy(
                    dst[:, (k * M + nm0) * P:(k * M + nm0 + w) * P],
                    st[:, om0 * P:(om0 + w) * P],
                )

    with tile.TileContext(nc) as tc:
        with (
            tc.tile_pool(name="persist", bufs=1) as persist_pool,
        ):
            # zx.T resident in SBUF: col = m*(T*8) + b*64 + t  (bf16, 128KB/par)
            zxT = persist_pool.tile([P, T * FB], bf16)
            zxT4 = zxT.rearrange("p (m b t) -> p m b t", m=M, b=B_LOC)
            b_sb = persist_pool.tile([P, M], f32)
            nc.sync.dma_start(b_sb[:], b_h.ap().rearrange("(m p) -> p m", p=P))
            # bf16 identity for the zx->PSUM injection matmul
            ident = persist_pool.tile([P, P], bf16)
            masks.make_identity(nc, ident[:])

            # ---------------- Phase A: zx.T = Wk.T @ x.T + b ----------------
            with (
                tc.tile_pool(name="wk", bufs=1) as wk_pool,
                tc.tile_pool(name="stage", bufs=2) as stage_pool,
                tc.tile_pool(name="nat", bufs=2) as nat_pool,
                tc.tile_pool(name="xtb", bufs=2) as xtb_pool,
                tc.tile_pool(name="gemm_psum", bufs=4, space="PSUM") as gps_pool,
            ):
                wk_sb = wk_pool.tile([P, KD * G], bf16)
                load_weight_bf16(wk_sb, wk_h, stage_pool)

                for chunk in range(T // TC):
                    t0 = chunk * TC
                    # natural x loads: tile bp holds rows (b=2bp..2bp+1, t0..t0+63)
                    natbs = []
                    for bp in range(4):
                        nat = nat_pool.tile([P, D], f32, name="nat", tag=f"nat{bp}")
                        for j in range(2):
                            nc.gpsimd.dma_start(
                                nat[j * TC:(j + 1) * TC, :],
                                x_ap[2 * bp + j, t0:t0 + TC, :],
                            )
                        natb = nat_pool.tile([P, D], bf16, name="natb", tag=f"natb{bp}")
                        nc.vector.tensor_copy(natb[:], nat[:])
                        natbs.append(natb)
                    # xbar transposes: xtb[k] cols = b*64 + t  (b-major)
                    xtbs = []
                    for k in range(KD):
                        xtb = xtb_pool.tile([P, TC * B_LOC], bf16,
                                            name=f"xtb{k}", tag=f"xtb{k}")
                        for bp in range(4):
                            nc.sync.dma_start(
                                xtb[:, bp * P:(bp + 1) * P],
                                natbs[bp][:, k * P:(k + 1) * P],
                                transpose=True,
                            )
                        xtbs.append(xtb)
                    for m in range(M):
                        ps = gps_pool.tile([P, TC * B_LOC], f32,
                                           name="gps", tag="gps")
                        for k in range(KD):
                            nc.tensor.matmul(
                                ps[:],
                                wk_sb[:, (k * M + m) * P:(k * M + m + 1) * P],
                                xtbs[k][:],
                                start=(k == 0),
                                stop=(k == KD - 1),
                            )
                        # copy-out + per-partition bias
                        # psum free = (b, t) b-major = contiguous dst slice
                        nc.scalar.activation(
                            zxT4[:, m, :, t0:t0 + TC],
                            ps.rearrange("p (b t) -> p b t", t=TC)[:],
                            AF.Identity,
                            bias=b_sb[:, PERMM[m]:PERMM[m] + 1],
                        )

            # ---------------- Phase B: the scan ----------------
            with (
                tc.tile_pool(name="wr", bufs=1) as wr_pool,
                tc.tile_pool(name="wstage2", bufs=2) as wstage2_pool,
                tc.tile_pool(name="state", bufs=1) as st_pool,
                tc.tile_pool(name="gates", bufs=2) as gate_pool,
                tc.tile_pool(name="tmp", bufs=2) as tmp_pool,
                tc.tile_pool(name="scan_psum", bufs=4, space="PSUM") as sps_pool,
            ):
                wr_sb = wr_pool.tile([P, KU * G], bf16)
                load_weight_bf16(wr_sb, wr_h, wstage2_pool)

                # h: bf16 per (parity, half); c: fp32 per (parity, half)
                hs = [[st_pool.tile([P, 2 * B_LOC], bf16, name=f"h{i}{j}")
                       for j in range(2)] for i in range(2)]
                cs = [[st_pool.tile([P, 2 * B_LOC], f32, name=f"c{i}{j}")
                       for j in range(2)] for i in range(2)]
                for j in range(2):
                    nc.vector.memset(hs[0][j][:], 0.0)
                    nc.vector.memset(cs[0][j][:], 0.0)
                hf = st_pool.tile([P, KU * B_LOC], f32, name="hf")

                # psum half tile col layout: a*16 + q*8 + b, a = gate class
                for t in range(time_steps):
                    pp = t % 2
                    qq = 1 - pp
                    h_prev = hs[pp]
                    last = t == time_steps - 1

                    pss = [sps_pool.tile([P, HB], f32, name=f"ps{hf_}",
                                         tag=f"ps{hf_}") for hf_ in range(2)]
                    zxh = [
                        (zxT4
                         .rearrange("p (a qq) b t -> p a qq b t", qq=4)
                         [:, :, 2 * half:2 * half + 2, :, t])
                        for half in range(2)
                    ]
                    # zx injection: psum <- I @ zx (start=True), off-chain
                    for half in range(2):
                        nc.tensor.matmul(
                            pss[half].rearrange("p (a q b) -> p a q b",
                                                q=2, b=B_LOC)[:],
                            ident[:],
                            zxh[half],
                            start=True,
                            stop=False,
                            skip_group_check=True,
                        )
                    # weight MMs, kk-major: kk{0,1} (need h half0 only) first,
                    # then kk{2,3} (h half1).  g-class m-tiles lead each
                    # segment so tanh(g) can run early on ACT.
                    for kpair in range(2):
                        for half in range(2):
                            for m in HALF_MS[half]:
                                a, q = m // 4, m % 4 - 2 * half
                                dst = pss[half][:, a * 16 + q * 8:
                                                a * 16 + q * 8 + 8]
                                for kk in (2 * kpair, 2 * kpair + 1):
                                    nc.tensor.matmul(
                                        dst,
                                        wr_sb[:, (kk * M + m) * P:
                                              (kk * M + m + 1) * P],
                                        h_prev[kk // 2][:, (kk % 2) * B_LOC:
                                                        (kk % 2 + 1) * B_LOC],
                                        start=False,
                                        stop=(kpair == 1 and kk == KU - 1
                                              and m == HALF_MS[half][-1]),
                                        skip_group_check=True,
                                    )

                    # gates: ACT order = expected ready order
                    gts = [gate_pool.tile([P, HB], f32, name=f"gt{half}",
                                          tag=f"gt{half}") for half in range(2)]
                    nc.scalar.activation(gts[0][:, 48:64], pss[0][:, 48:64],
                                         AF.Tanh)
                    nc.scalar.activation(gts[0][:, 0:48], pss[0][:, 0:48],
                                         AF.Sigmoid)
                    nc.scalar.activation(gts[1][:, 48:64], pss[1][:, 48:64],
                                         AF.Tanh)
                    nc.scalar.activation(gts[1][:, 0:48], pss[1][:, 0:48],
                                         AF.Sigmoid)

                    t1s, t2s, tcs = [], [], []
                    for half in range(2):
                        gt = gts[half]
                        # all elementwise muls/adds on DVE (GPSIMD sw ops
                        # measured ~930ns each on hw -- far too slow)
                        t1 = tmp_pool.tile([P, 2 * B_LOC], f32,
                                           name=f"t1{half}", tag=f"t1{half}")
                        nc.vector.tensor_mul(t1[:], gt[:, 16:32], cs[pp][half][:])
                        t2 = tmp_pool.tile([P, 2 * B_LOC], f32,
                                           name=f"t2{half}", tag=f"t2{half}")
                        nc.vector.tensor_mul(t2[:], gt[:, 0:16], gt[:, 48:64])
                        nc.vector.tensor_add(cs[qq][half][:], t1[:], t2[:])
                        tc_t = tmp_pool.tile([P, 2 * B_LOC], f32,
                                             name=f"tc{half}", tag=f"tc{half}")
                        nc.scalar.activation(tc_t[:], cs[qq][half][:], AF.Tanh)
                        t1s.append(t1)
                        t2s.append(t2)
                        tcs.append(tc_t)
                    for half in range(2):
                        if last:
                            nc.vector.tensor_mul(
                                hf[:, half * 16:(half + 1) * 16],
                                gts[half][:, 32:48], tcs[half][:],
                            )
                        else:
                            nc.vector.tensor_mul(hs[qq][half][:],
                                                 gts[half][:, 32:48],
                                                 tcs[half][:])

                for kk in range(KU):
                    nc.sync.dma_start(
                        out_h.ap()[:, kk * P:(kk + 1) * P].rearrange("b p -> p b"),
                        hf[:, kk * B_LOC:(kk + 1) * B_LOC],
                    )

    nc.compile()
    return nc


def _get_nc(time_steps=T):
    key = time_steps
    if key not in _CACHE:
        _CACHE[key] = _build(time_steps)
    return _CACHE[key]


def kernel(x, Wk, Wr, b):
    from concourse import bass_utils

    x = np.ascontiguousarray(np.asarray(x, dtype=np.float32))
    Wk = np.ascontiguousarray(np.asarray(Wk, dtype=np.float32))
    Wr = np.ascontiguousarray(np.asarray(Wr, dtype=np.float32))
    b = np.ascontiguousarray(np.asarray(b, dtype=np.float32))

    nc = _get_nc(T)
    in_maps = [
        {
            "x": x[c * B_LOC:(c + 1) * B_LOC],
            "Wk": Wk,
            "Wr": Wr,
            "b": b,
        }
        for c in range(N_CORES)
    ]
    res = bass_utils.run_bass_kernel_spmd(nc, in_maps, core_ids=list(range(N_CORES)))
    return np.concatenate([res.results[c]["h_last"] for c in range(N_CORES)], axis=0)
